# revision 1
# baseline (speedup 1.0000x reference)
"""Trainium2 Bass kernel for nn_BaseEncoder (ragged entity-pair encoder).

Contract: kernel(**inputs) takes the FULL unsharded inputs (numpy) and
returns the FULL output [B, Q, E, E, R] float32.

Sharding: B*Q = 8 independent (batch, query) pairs -> one per NeuronCore.
Small weights (W_head / W_tail / prototypes-for-that-b) are replicated.

Host-side prep per core (cheap, index/layout only):
  - gather the E*M mention rows of the per-query attention and sum over the
    M=2 mentions (the /2 and /NH scalings cancel in the later row-softmax-
    style normalization, so they are dropped),
  - transpose to At[l, (h,e)] so the device never needs a transpose,
  - entity means ent = mean_m seq[pos] (transposed to entT),
  - prototypes for this b, reshaped/transposed to [2H, R*P].

Device kernel per core (all fp32):
  mul[l, e*32+f] = sum_h At[l,h,e] * At[l,h,f]              (VectorE)
  S[ef]   = sum_l mul[l, ef]                                 (TensorE, ones)
  ctxT[h', ef] = sum_l seq[l, h'] * mul[l, ef]               (TensorE)
  ctxnT = ctxT * (1/S)                                       (VectorE)
  epH[h'', e] = sum_h' W_head[h', h''] entT[h', e]  (and tail)    (TensorE)
  hT[h'', ef] = tanh(sum_h' W_head[768+h', h''] ctxnT[h', ef] + epH[h'', e])
  tT[h'', ef] = tanh(... W_tail ... + epT[h'', f])       (TensorE+VectorE+ScalarE)
  scores[ef, rp] = sum_d candT[d, ef] * protoT[d, rp]        (TensorE)
  out[ef, r] = max_p scores[ef, r*10+p]                      (VectorE)
"""

import numpy as np

B, Q, L, H, E, M, R, P, NH = 2, 4, 1024, 768, 32, 2, 5, 10, 12
NCORES = 8
LT = L // 128          # 8 l-tiles
HT = H // 128          # 6 tiles of 128 along a hidden dim
EF = E * E             # 1024 entity pairs
RP = R * P             # 50 prototype rows

_CACHE = {}


def _build_program():
    import concourse.mybir as mybir
    import concourse.tile as tile
    from concourse import bacc

    f32 = mybir.dt.float32
    nc = bacc.Bacc("TRN2", target_bir_lowering=False, debug=False,
                   num_devices=NCORES)

    at_d = nc.dram_tensor("at", [L, NH * E], f32, kind="ExternalInput").ap()
    seq_d = nc.dram_tensor("seq", [L, H], mybir.dt.float32r, kind="ExternalInput").ap()
    entT_d = nc.dram_tensor("entT", [H, E], f32, kind="ExternalInput").ap()
    wh_d = nc.dram_tensor("wh", [2 * H, H], mybir.dt.float32r, kind="ExternalInput").ap()
    wt_d = nc.dram_tensor("wt", [2 * H, H], mybir.dt.float32r, kind="ExternalInput").ap()
    ptT_d = nc.dram_tensor("ptT", [2 * H, RP], mybir.dt.float32r, kind="ExternalInput").ap()
    out_d = nc.dram_tensor("out", [EF, R], f32, kind="ExternalOutput").ap()

    with tile.TileContext(nc) as tc:
        _emit(tc, mybir, at_d, seq_d, entT_d, wh_d, wt_d, ptT_d, out_d)

    nc.compile()
    return nc


USE_F32R = True
HC = EF // 2            # 512-wide ef chunk (= one PSUM bank of fp32)


def _emit(tc, mybir, at_d, seq_d, entT_d, wh_d, wt_d, ptT_d, out_d):
    nc = tc.nc
    f32 = mybir.dt.float32
    f32r = mybir.dt.float32r

    Alu = mybir.AluOpType
    Act = mybir.ActivationFunctionType
    Ax = mybir.AxisListType
    from concourse.masks import make_identity

    import contextlib
    ctx = contextlib.ExitStack()
    with ctx:
        const = ctx.enter_context(tc.tile_pool(name="const", bufs=1))
        big = ctx.enter_context(tc.tile_pool(name="big", bufs=1))
        mulp = ctx.enter_context(tc.tile_pool(name="mulp", bufs=12))
        candp = ctx.enter_context(tc.tile_pool(name="candp", bufs=14))
        ctxp = ctx.enter_context(tc.tile_pool(name="ctxp", bufs=2))
        tmp = ctx.enter_context(tc.tile_pool(name="tmp", bufs=3))
        # PSUM: 8 banks statically split into tags
        #   "ctx": 6 x 1 bank   (per-chunk ctx accumulators; later proj-B)
        #   "sg":  1 x 1 bank   (S-gram, recS broadcast, even proj-A groups)
        #   "tail": 1 x 1 bank  (ep, odd proj-A groups, scores, transposes)
        psum = ctx.enter_context(tc.tile_pool(name="psum", bufs=1, space="PSUM"))

        # ---------------- input loads ----------------
        at_sb = big.tile([128, LT, NH * E], f32, tag="at_sb")
        at_r = at_d.rearrange("(t p) n -> p t n", p=128)
        for lt in range(LT):
            nc.sync.dma_start(out=at_sb[:, lt, :], in_=at_r[:, lt, :])
        seq_sb = big.tile([128, LT, H], f32r, tag="seq_sb")
        nc.sync.dma_start(out=seq_sb, in_=seq_d.rearrange("(t p) n -> p t n", p=128))
        entT_sb = const.tile([128, HT, E], f32, tag="entT_sb")
        nc.sync.dma_start(out=entT_sb, in_=entT_d.rearrange("(t p) n -> p t n", p=128))
        ptT_sb = const.tile([128, 2 * HT, RP], f32r, tag="ptT_sb")
        nc.sync.dma_start(out=ptT_sb, in_=ptT_d.rearrange("(t p) n -> p t n", p=128))
        wh_sb = big.tile([128, 2 * HT, H], f32r, tag="wh_sb")
        nc.sync.dma_start(out=wh_sb, in_=wh_d.rearrange("(t p) n -> p t n", p=128))
        wt_sb = big.tile([128, 2 * HT, H], f32r, tag="wt_sb")
        nc.sync.dma_start(out=wt_sb, in_=wt_d.rearrange("(t p) n -> p t n", p=128))

        ones_row = const.tile([1, 128], f32, tag="ones_row")
        nc.vector.memset(ones_row, 1.0)
        ident = const.tile([RP, RP], f32, tag="ident")
        make_identity(nc, ident)
        recd = nc.dram_tensor("recd", [E, E], f32).ap()

        # ------- S via Gram over the raw At slices (independent of mul) ----
        # S[e, f] = sum_{h, l} At[l, (h, e)] * At[l, (h, f)]
        sg_ps = psum.tile([E, E], f32, tag="sg", bufs=1, name="sg_ps")
        n_acc = LT * NH
        k = 0
        for lt in range(LT):
            for h in range(NH):
                sl = at_sb[:, lt, h * E:(h + 1) * E]
                nc.tensor.matmul(sg_ps, sl, sl, start=(k == 0),
                                 stop=(k == n_acc - 1))
                k += 1
        r2_sb = const.tile([E, E], f32, tag="r2_sb")
        nc.scalar.copy(r2_sb, sg_ps)
        nc.vector.reciprocal(r2_sb, r2_sb)
        # flatten [32, 32] -> [1, 1024] via a DRAM bounce, then broadcast to
        # all 128 partitions with ones[1,128].T @ chunk.
        nc.sync.dma_start(out=recd, in_=r2_sb)
        rec1 = const.tile([1, EF], f32, tag="rec1")
        nc.sync.dma_start(out=rec1,
                          in_=recd.rearrange("a b -> (a b)")[None, :])
        recS_sb = big.tile([128, EF], f32, tag="recS_sb")
        for c in range(2):
            rb = psum.tile([128, HC], f32, tag="sg", bufs=1, name="recB")
            nc.tensor.matmul(rb, ones_row, rec1[:, c * HC:(c + 1) * HC],
                             start=True, stop=True)
            nc.scalar.copy(recS_sb[:, c * HC:(c + 1) * HC], rb)

        # ---------------- entity projections (ent @ W[:H]) ----------------
        ep_sb = []
        for w, wsb in ((0, wh_sb), (1, wt_sb)):
            ep = const.tile([128, HT, E], f32, tag=f"ep{w}", name=f"ep{w}")
            ep_sb.append(ep)
            for ht2 in range(HT):
                ps = psum.tile([128, E], f32, tag="tail", bufs=1, name="ep_ps")
                for kt in range(HT):
                    nc.tensor.matmul(
                        ps, wsb[:, kt, ht2 * 128:(ht2 + 1) * 128].bitcast(f32),
                        entT_sb[:, kt, :],
                        start=(kt == 0), stop=(kt == HT - 1))
                nc.scalar.copy(ep[:, ht2, :], ps)

        # ---------------- chunked main pipeline ----------------
        # Chunk c covers pairs ef in [c*512, (c+1)*512) i.e. e in [16c, 16c+16).
        EC = E // 2

        def emit_mul_chunk(c, lt, mulA=None):
            """VectorE: mul products+adds for chunk c, l-tile lt.

            Chunk 1 exploits symmetry: its f<16 half equals the transpose of
            chunk 0's f>=16 half, so only the (e>=16, f>=16) quadrant is
            computed; the rest is one strided copy from the chunk-0 tile.
            """
            at3 = at_sb[:, lt, :].rearrange("p (h e) -> p h e", h=NH)
            mt = mulp.tile([128, HC], f32r, tag="mul", name=f"mul{c}_{lt}")
            m3 = mt.rearrange("p (e f) -> p e f", e=EC)
            es = c * EC
            fs = 0 if c == 0 else EC
            FW = E - fs
            for h in range(NH):
                a_e = at3[:, h, es:es + EC, None].broadcast_to([128, EC, FW])
                a_f = at3[:, h, None, fs:].broadcast_to([128, EC, FW])
                if h == 0:
                    nc.vector.tensor_mul(m3[:, :, fs:], a_e, a_f)
                else:
                    t = tmp.tile([128, EC, E], f32, tag="scratch",
                                 name="prod")
                    tq = t[:, :, :FW]
                    nc.vector.tensor_mul(tq, a_e, a_f)
                    nc.vector.tensor_add(m3[:, :, fs:], m3[:, :, fs:], tq)
            if c == 1:
                # m3[e2, f1] = mulA[f1, 16+e2] for f1 < 16 (Gram symmetry)
                w = mulA.rearrange("p (e f) -> p e f", e=EC)[:, :, EC:]
                nc.vector.tensor_copy(m3[:, :, :EC],
                                      w.rearrange("p a b -> p b a"))
            return mt

        def emit_ctx_chunk(c, lt, mt, ctx_ps):
            for ht in range(HT):
                nc.tensor.matmul(
                    ctx_ps[ht], seq_sb[:, lt, ht * 128:(ht + 1) * 128],
                    mt, start=(lt == 0), stop=(lt == LT - 1))

        def emit_norm_chunk(c, ctx_ps):
            cn = ctxp.tile([128, HT, HC], f32r, tag="ctxn", name=f"ctxn{c}")
            for ht in range(HT):
                nc.vector.tensor_mul(cn[:, ht, :], ctx_ps[ht],
                                     recS_sb[:, c * HC:(c + 1) * HC])
            return cn

        def emit_proj_group(c, g, cn, cand_t, ps_tag):
            w, ht2 = divmod(g, HT)
            wsb = wh_sb if w == 0 else wt_sb
            nb = HT if ps_tag == "ctx" else 1
            ps = psum.tile([128, HC], f32, tag=ps_tag, bufs=nb,
                           name=f"proj{c}_{g}")
            for kt in range(HT):
                nc.tensor.matmul(ps, wsb[:, HT + kt, ht2 * 128:(ht2 + 1) * 128],
                                 cn[:, kt, :],
                                 start=(kt == 0), stop=(kt == HT - 1))
            es = c * EC
            if w == 0:
                bias = ep_sb[0][:, ht2, es:es + EC, None].broadcast_to(
                    [128, EC, E])
            else:
                bias = ep_sb[1][:, ht2, None, :].broadcast_to([128, EC, E])
            pre = tmp.tile([128, EC, E], f32, tag="scratch", name="pre")
            nc.vector.tensor_add(pre, ps.rearrange("p (e f) -> p e f", e=EC),
                                 bias)
            cd = candp.tile([128, HC], f32r, tag="cand", name=f"cand{c}_{g}")
            cand_t[g] = cd
            nc.scalar.activation(cd, pre.rearrange("p a b -> p (a b)"),
                                 Act.Tanh)

        def emit_scores_chunk(c, cand_t, ps_tag):
            sc = psum.tile([RP, HC], f32, tag=ps_tag, bufs=1, name=f"sc{c}")
            order = [w * HT + kt for w in range(2) for kt in range(HT)]
            for i, g in enumerate(order):
                nc.tensor.matmul(sc, ptT_sb[:, g, :], cand_t[g],
                                 start=(i == 0), stop=(i == 2 * HT - 1))
            scT = const.tile([RP, HC], f32, tag=f"scT{c}", name=f"scT{c}")
            nc.scalar.copy(scT, sc)
            ob = const.tile([128, LT // 2, R], f32, tag=f"ob{c}",
                            name=f"ob{c}")
            for et in range(LT // 2):
                tp = psum.tile([128, RP], f32, tag="sg", bufs=1, name="tp")
                nc.tensor.transpose(tp, scT[:, et * 128:(et + 1) * 128],
                                    ident)
                nc.vector.tensor_reduce(
                    out=ob[:, et, :],
                    in_=tp.rearrange("p (r q) -> p r q", r=R),
                    axis=Ax.X, op=Alu.max)
            nc.sync.dma_start(
                out=out_d.rearrange("(t p) r -> p t r", p=128)[
                    :, c * (LT // 2):(c + 1) * (LT // 2), :],
                in_=ob)

        # ---- phase A: mul+ctx for chunk 0 ----
        ctxA_ps = [psum.tile([128, HC], f32, tag="ctx", bufs=HT,
                             name=f"ctxA{ht}") for ht in range(HT)]
        mulA_t = []
        for lt in range(LT):
            mt = emit_mul_chunk(0, lt)
            mulA_t.append(mt)
            emit_ctx_chunk(0, lt, mt, ctxA_ps)
        cnA = emit_norm_chunk(0, ctxA_ps)

        # ---- phase B: mul+ctx for chunk 1, interleaved with chunk-0 tail ---
        candA = [None] * (2 * HT)
        ctxB_ps = [psum.tile([128, HC], f32, tag="ctx", bufs=HT,
                             name=f"ctxB{ht}") for ht in range(HT)]
        projA_sched = {1: [0, 1], 2: [2, 3], 3: [4, 5], 4: [6, 7],
                       5: [8, 9], 6: [10, 11]}
        for lt in range(LT):
            mt = emit_mul_chunk(1, lt, mulA=mulA_t[lt])
            emit_ctx_chunk(1, lt, mt, ctxB_ps)
            for g in projA_sched.get(lt, []):
                emit_proj_group(0, g, cnA, candA, "sg" if g % 2 == 0
                                else "tail")
        emit_scores_chunk(0, candA, "tail")
        cnB = emit_norm_chunk(1, ctxB_ps)

        # ---- chunk-1 tail (PE slots from the freed ctx accumulators) ----
        candB = [None] * (2 * HT)
        for g in range(2 * HT):
            emit_proj_group(1, g, cnB, candB, "ctx")
        emit_scores_chunk(1, candB, "tail")


def _host_prep(sequence_output, attention, W_head, W_tail, prototypes,
               mention_pos):
    """Build the per-core input maps (numpy only)."""
    seq = np.ascontiguousarray(sequence_output, dtype=np.float32)
    att = np.asarray(attention, dtype=np.float32)
    wh = np.ascontiguousarray(W_head, dtype=np.float32)
    wt = np.ascontiguousarray(W_tail, dtype=np.float32)
    pro = np.asarray(prototypes, dtype=np.float32)
    pos = np.asarray(mention_pos)

    in_maps = []
    for c in range(NCORES):
        b, q = divmod(c, Q)
        p_bq = pos[b, q]                       # [E, M]
        # attention gather + mention-sum: [NH, E, L] (scale dropped)
        g = att[b, q][:, p_bq, :]              # [NH, E, M, L]
        asum = g[:, :, 0, :] + g[:, :, 1, :]   # [NH, E, L]
        at = np.ascontiguousarray(
            asum.reshape(NH * E, L).T)         # [L, NH*E], At[l, h*E+e]
        # entity means: [E, H] -> entT [H, E]
        ment = seq[b, q][p_bq]                 # [E, M, H]
        ent = (ment[:, 0, :] + ment[:, 1, :]) * np.float32(0.5)
        entT = np.ascontiguousarray(ent.T)
        ptT = np.ascontiguousarray(
            pro[b].reshape(RP, 2 * H).T)       # [2H, RP]
        in_maps.append({
            "at": at,
            "seq": seq[b, q],
            "entT": entT,
            "wh": wh,
            "wt": wt,
            "ptT": ptT,
        })
    return in_maps


def kernel(sequence_output, attention, W_head, W_tail, prototypes,
           mention_pos):
    from concourse.bass_utils import run_bass_kernel_spmd

    if "nc" not in _CACHE:
        _CACHE["nc"] = _build_program()
    nc = _CACHE["nc"]

    in_maps = _host_prep(sequence_output, attention, W_head, W_tail,
                         prototypes, mention_pos)
    res = run_bass_kernel_spmd(nc, in_maps, core_ids=list(range(NCORES)))

    out = np.empty((B, Q, E, E, R), dtype=np.float32)
    for c in range(NCORES):
        b, q = divmod(c, Q)
        out[b, q] = res.results[c]["out"].reshape(E, E, R)
    return out



# revision 2
# speedup vs baseline: 1.6570x; 1.6570x over previous
"""Trainium2 Bass kernel for nn_BaseEncoder (ragged entity-pair encoder).

Contract: kernel(**inputs) takes the FULL unsharded inputs (numpy) and
returns the FULL output [B, Q, E, E, R] float32.

Sharding: B*Q = 8 independent (batch, query) pairs -> one per NeuronCore.
Small weights (W_head / W_tail / prototypes-for-that-b) are replicated.

Host-side prep per core (cheap, index/layout only):
  - gather the E*M mention rows of the per-query attention and sum over the
    M=2 mentions (the /2 and /NH scalings cancel in the later row-softmax-
    style normalization, so they are dropped),
  - transpose to At[l, (h,e)] so the device never needs a transpose,
  - S[e,f] = sum_{l,h} At[l,h,e]*At[l,h,f] and recs = 16/S (the 16 keeps
    recs in fp16 normal range; compensated by scaling W[H:] rows by 1/16),
  - entity means ent = mean_m seq[pos] (transposed to entT),
  - prototypes for this b, reshaped/transposed to [2H, R*P].

Device kernel per core (fp16 data, fp32 PSUM accumulation):
  prod[l,h,e,f] = At[l,h,e]*At[l,h,f]   (VectorE, fused packed-pair 2x op)
  mul[l,ef] = sum_h prod                 (VectorE tree adds, 2x)
  ctxT[h',ef] = sum_l seq[l,h'] mul[l,ef]          (TensorE)
  cn = ctxT * recs                                  (ScalarE copy + VectorE)
  epT[e,h''] = sum_h' entT[h',e] W[h',h'']          (TensorE)
  pre[h'',ef] = sum_kt W[H+kt,h''] cn[kt,ef] + mask-fold of epT   (TensorE)
  cand = tanh(pre)                                  (ScalarE, from PSUM)
  scores[rp,ef] = sum_d candT[d,ef] protoT[d,rp]    (TensorE)
  out[ef,r] = max_p scores                          (transpose + VectorE)
"""

import numpy as np

B, Q, L, H, E, M, R, P, NH = 2, 4, 1024, 768, 32, 2, 5, 10, 12
NCORES = 8
LT = L // 128          # 8 l-tiles
HT = H // 128          # 6 tiles of 128 along a hidden dim
EF = E * E             # 1024 entity pairs
RP = R * P             # 50 prototype rows
EC = E // 2            # 16 e-rows per chunk
HC = EF // 2           # 512-wide ef chunk (= one PSUM bank of fp32)

_CACHE = {}


def _build_program():
    import concourse.mybir as mybir
    import concourse.tile as tile
    from concourse import bacc

    f16 = mybir.dt.float16
    f32 = mybir.dt.float32
    nc = bacc.Bacc("TRN2", target_bir_lowering=False, debug=False,
                   num_devices=NCORES)

    at_d = nc.dram_tensor("at", [L, NH * E], f16, kind="ExternalInput").ap()
    seq_d = nc.dram_tensor("seq", [L, H], f16, kind="ExternalInput").ap()
    entT_d = nc.dram_tensor("entT", [H, E], f16, kind="ExternalInput").ap()
    wh_d = nc.dram_tensor("wh", [2 * H, H], f16, kind="ExternalInput").ap()
    wt_d = nc.dram_tensor("wt", [2 * H, H], f16, kind="ExternalInput").ap()
    ptT_d = nc.dram_tensor("ptT", [2 * H, RP], f16, kind="ExternalInput").ap()
    recs_d = nc.dram_tensor("recs", [1, EF], f16, kind="ExternalInput").ap()
    out_d = nc.dram_tensor("out", [EF, R], f32, kind="ExternalOutput").ap()

    with tile.TileContext(nc) as tc:
        _emit(tc, mybir, at_d, seq_d, entT_d, wh_d, wt_d, ptT_d, recs_d,
              out_d)

    nc.compile()
    return nc


def _emit(tc, mybir, at_d, seq_d, entT_d, wh_d, wt_d, ptT_d, recs_d, out_d):
    nc = tc.nc
    f16 = mybir.dt.float16
    f32 = mybir.dt.float32

    Alu = mybir.AluOpType
    Act = mybir.ActivationFunctionType
    Ax = mybir.AxisListType
    from concourse.masks import make_identity

    import contextlib
    ctx = contextlib.ExitStack()
    with ctx:
        const = ctx.enter_context(tc.tile_pool(name="const", bufs=1))
        big = ctx.enter_context(tc.tile_pool(name="big", bufs=1))
        mulp = ctx.enter_context(tc.tile_pool(name="mulp", bufs=12))
        candp = ctx.enter_context(tc.tile_pool(name="candp", bufs=14))
        ctxp = ctx.enter_context(tc.tile_pool(name="ctxp", bufs=2))
        tmp = ctx.enter_context(tc.tile_pool(name="tmp", bufs=1))
        arep2p = ctx.enter_context(tc.tile_pool(name="arep2p", bufs=3))
        # PSUM: 8 banks statically split into tags
        #   "ctx": 6 x 1 bank   (per-chunk ctx accumulators; later proj-B)
        #   "sg":  1 x 1 bank   (even proj-A groups, transposes)
        #   "tail": 1 x 1 bank  (epT, odd proj-A groups, scores)
        psum = ctx.enter_context(tc.tile_pool(name="psum", bufs=1,
                                              space="PSUM"))

        # ---------------- input loads ----------------
        at_sb = big.tile([128, LT, NH * E], f16, tag="at_sb")
        at_r = at_d.rearrange("(t p) n -> p t n", p=128)
        for lt in range(LT):
            nc.sync.dma_start(out=at_sb[:, lt, :], in_=at_r[:, lt, :])
        entT_sb = const.tile([128, HT, E], f16, tag="entT_sb")
        nc.sync.dma_start(out=entT_sb, in_=entT_d.rearrange(
            "(t p) n -> p t n", p=128))
        seq_sb = big.tile([128, LT, H], f16, tag="seq_sb")
        nc.sync.dma_start(out=seq_sb, in_=seq_d.rearrange(
            "(t p) n -> p t n", p=128))
        # recs broadcast to all 128 partitions straight from the DMA
        recS_sb = big.tile([128, EF], f16, tag="recS_sb")
        nc.sync.dma_start(out=recS_sb, in_=recs_d.partition_broadcast(128))
        wh_sb = big.tile([128, 2 * HT, H], f16, tag="wh_sb")
        nc.sync.dma_start(out=wh_sb, in_=wh_d.rearrange(
            "(t p) n -> p t n", p=128))
        wt_sb = big.tile([128, 2 * HT, H], f16, tag="wt_sb")
        nc.sync.dma_start(out=wt_sb, in_=wt_d.rearrange(
            "(t p) n -> p t n", p=128))
        ptT_sb = const.tile([128, 2 * HT, RP], f16, tag="ptT_sb")
        nc.sync.dma_start(out=ptT_sb, in_=ptT_d.rearrange(
            "(t p) n -> p t n", p=128))

        # ---------------- constants: identities and bias masks ----------
        ident32 = const.tile([E, E], f16, tag="ident32")
        make_identity(nc, ident32)
        identRP = const.tile([RP, RP], f32, tag="identRP")
        make_identity(nc, identRP)
        # mask_h[c][e', (el,f)] = 1 iff e' == 16c+el ; mask_t[f',(el,f)] =
        # 1 iff f'==f. Rows >= 32 are zero so the (garbage-free) epT
        # stationary rows beyond 32 contribute nothing.
        mask_h = []
        for c in range(2):
            mk = const.tile([128, HC], f16, tag=f"mask_h{c}")
            nc.gpsimd.memset(mk, 0.0)
            nc.vector.tensor_copy(
                mk[0:E, :].rearrange("p (e f) -> p e f", e=EC),
                ident32[:, c * EC:(c + 1) * EC, None].broadcast_to(
                    [E, EC, E]))
            mask_h.append(mk)
        mask_t = const.tile([128, HC], f16, tag="mask_t")
        nc.gpsimd.memset(mask_t, 0.0)
        nc.vector.tensor_copy(
            mask_t[0:E, :].rearrange("p (e f) -> p e f", e=EC),
            ident32[:, None, :].broadcast_to([E, EC, E]))

        # ---------------- entity projections epT[e, h''] ------------------
        # epT_w = entT^T(W_w[:H]) : stationary entT [h'-part, e], moving W.
        epT_sb = const.tile([128, 2, H], f16, tag="epT_sb")
        nc.gpsimd.memset(epT_sb, 0.0)

        def emit_epT():
            HH = H // 2
            for w, wsb in ((0, wh_sb), (1, wt_sb)):
                for half in range(2):
                    ps = psum.tile([E, HH], f32, tag="tail", bufs=1,
                                   name=f"epT{w}_{half}")
                    for kt in range(HT):
                        nc.tensor.matmul(
                            ps, entT_sb[:, kt, :],
                            wsb[:, kt, half * HH:(half + 1) * HH],
                            start=(kt == 0), stop=(kt == HT - 1))
                    nc.scalar.copy(
                        epT_sb[0:E, w, half * HH:(half + 1) * HH], ps)

        # ---------------- chunked main pipeline ----------------
        # Chunk c covers pairs ef in [c*512, (c+1)*512) i.e. e in
        # [16c, 16c+16).  prod[l,h,e,f] computed as packed fp16 pairs so the
        # DVE runs in 2x mode; h-sum via tree adds.

        def emit_arep2(c, lt):
            """Act engine: materialize at[l,h,e] duplicated in f-pairs."""
            at3 = at_sb[:, lt, :].rearrange("p (h e) -> p h e", h=NH)
            a2 = arep2p.tile([128, NH, EC, 2], f16, tag="arep2",
                             name=f"arep2_{c}_{lt}")
            nc.scalar.copy(
                a2, at3[:, :, c * EC:(c + 1) * EC, None].broadcast_to(
                    [128, NH, EC, 2]))
            return a2

        def emit_mul_chunk(c, lt, a2, mulA=None):
            at3 = at_sb[:, lt, :].rearrange("p (h e) -> p h e", h=NH)
            mt = mulp.tile([128, HC], f16, tag="mul", name=f"mul{c}_{lt}")
            fs = 0 if c == 0 else EC
            FW = E - fs
            # products: out[p,h,e,fh,fl] = a2[p,h,e,fl] * at3[p,h,(fh fl)]
            pr = tmp.tile([128, NH, EC, FW], f16, tag=f"prod{c}",
                          name=f"prod{c}_{lt}")
            in1 = a2[:, :, :, None, :].broadcast_to(
                [128, NH, EC, FW // 2, 2])
            in2 = at3[:, :, fs:].rearrange(
                "p h (fh fl) -> p h fh fl", fl=2)[:, :, None, :, :]
            in2 = in2.broadcast_to([128, NH, EC, FW // 2, 2])
            nc.vector.tensor_mul(
                pr.rearrange("p h e (fh fl) -> p h e fh fl", fl=2), in1, in2)
            # h-sum tree: 12 -> 6 -> 3 -> out
            nc.vector.tensor_add(pr[:, 0:6], pr[:, 0:6], pr[:, 6:12])
            nc.vector.tensor_add(pr[:, 0:3], pr[:, 0:3], pr[:, 3:6])
            m3 = mt.rearrange("p (e f) -> p e f", e=EC)
            nc.vector.tensor_add(m3[:, :, fs:], pr[:, 0], pr[:, 1])
            nc.vector.tensor_add(m3[:, :, fs:], m3[:, :, fs:], pr[:, 2])
            if c == 1:
                # m3[e2, f1] = mulA[f1, 16+e2] for f1 < 16 (Gram symmetry)
                w = mulA.rearrange("p (e f) -> p e f", e=EC)[:, :, EC:]
                nc.scalar.copy(m3[:, :, :EC],
                               w.rearrange("p a b -> p b a"))
            return mt

        def emit_ctx_chunk(c, lt, mt, ctx_ps):
            for ht in range(HT):
                nc.tensor.matmul(
                    ctx_ps[ht], seq_sb[:, lt, ht * 128:(ht + 1) * 128],
                    mt, start=(lt == 0), stop=(lt == LT - 1))

        def emit_norm_chunk(c, ctx_ps):
            cn = ctxp.tile([128, HT, HC], f16, tag="ctxn", name=f"ctxn{c}")
            cc = tmp.tile([128, HT, HC], f16, tag="ctxc", name=f"ctxc{c}")
            for ht in range(HT):
                nc.scalar.copy(cc[:, ht, :], ctx_ps[ht])
                nc.vector.tensor_mul(cn[:, ht, :], cc[:, ht, :],
                                     recS_sb[:, c * HC:(c + 1) * HC])
            return cn

        def emit_proj_group(c, g, cn, cand_t, ps_tag):
            w, ht2 = divmod(g, HT)
            wsb = wh_sb if w == 0 else wt_sb
            nb = HT if ps_tag == "ctx" else 1
            ps = psum.tile([128, HC], f32, tag=ps_tag, bufs=nb,
                           name=f"proj{c}_{g}")
            for kt in range(HT):
                nc.tensor.matmul(ps, wsb[:, HT + kt,
                                         ht2 * 128:(ht2 + 1) * 128],
                                 cn[:, kt, :],
                                 start=(kt == 0), stop=False)
            # bias fold: += epT_w[sel(ef), h''] via the 0/1 mask moving
            mk = mask_h[c] if w == 0 else mask_t
            nc.tensor.matmul(ps, epT_sb[:, w, ht2 * 128:(ht2 + 1) * 128],
                             mk, start=False, stop=True)
            cd = candp.tile([128, HC], f16, tag="cand", name=f"cand{c}_{g}")
            cand_t[g] = cd
            nc.scalar.activation(cd, ps, Act.Tanh)

        def emit_scores_chunk(c, cand_t, ps_tag):
            sc = psum.tile([RP, HC], f32, tag=ps_tag, bufs=1, name=f"sc{c}")
            order = [w * HT + kt for w in range(2) for kt in range(HT)]
            for i, g in enumerate(order):
                nc.tensor.matmul(sc, ptT_sb[:, g, :], cand_t[g],
                                 start=(i == 0), stop=(i == 2 * HT - 1))
            scT = const.tile([RP, HC], f32, tag=f"scT{c}", name=f"scT{c}")
            nc.scalar.copy(scT, sc)
            ob = const.tile([128, LT // 2, R], f32, tag=f"ob{c}",
                            name=f"ob{c}")
            for et in range(LT // 2):
                tp = psum.tile([128, RP], f32, tag="sg", bufs=1, name="tp")
                nc.tensor.transpose(tp, scT[:, et * 128:(et + 1) * 128],
                                    identRP)
                nc.vector.tensor_reduce(
                    out=ob[:, et, :],
                    in_=tp.rearrange("p (r q) -> p r q", r=R),
                    axis=Ax.X, op=Alu.max)
            nc.sync.dma_start(
                out=out_d.rearrange("(t p) r -> p t r", p=128)[
                    :, c * (LT // 2):(c + 1) * (LT // 2), :],
                in_=ob)

        # ---- phase A: mul+ctx for chunk 0 ----
        ctxA_ps = [psum.tile([128, HC], f32, tag="ctx", bufs=HT,
                             name=f"ctxA{ht}") for ht in range(HT)]
        mulA_t = []
        for lt in range(LT):
            a2 = emit_arep2(0, lt)
            mt = emit_mul_chunk(0, lt, a2)
            mulA_t.append(mt)
            emit_ctx_chunk(0, lt, mt, ctxA_ps)
            if lt == 2:
                emit_epT()
        cnA = emit_norm_chunk(0, ctxA_ps)

        # ---- phase B: mul+ctx for chunk 1, interleaved with chunk-0 tail ---
        candA = [None] * (2 * HT)
        ctxB_ps = [psum.tile([128, HC], f32, tag="ctx", bufs=HT,
                             name=f"ctxB{ht}") for ht in range(HT)]
        projA_sched = {1: [0, 1], 2: [2, 3], 3: [4, 5], 4: [6, 7],
                       5: [8, 9], 6: [10, 11]}
        for lt in range(LT):
            a2 = emit_arep2(1, lt)
            mt = emit_mul_chunk(1, lt, a2, mulA=mulA_t[lt])
            emit_ctx_chunk(1, lt, mt, ctxB_ps)
            for g in projA_sched.get(lt, []):
                emit_proj_group(0, g, cnA, candA, "sg" if g % 2 == 0
                                else "tail")
        emit_scores_chunk(0, candA, "tail")
        cnB = emit_norm_chunk(1, ctxB_ps)

        # ---- chunk-1 tail (PE slots from the freed ctx accumulators) ----
        candB = [None] * (2 * HT)
        for g in range(2 * HT):
            emit_proj_group(1, g, cnB, candB, "ctx")
        emit_scores_chunk(1, candB, "tail")


def _host_prep(sequence_output, attention, W_head, W_tail, prototypes,
               mention_pos):
    """Build the per-core input maps (numpy only)."""
    seq = np.asarray(sequence_output, dtype=np.float32)
    att = np.asarray(attention, dtype=np.float32)
    wh = np.asarray(W_head, dtype=np.float32).copy()
    wt = np.asarray(W_tail, dtype=np.float32).copy()
    # the device normalizer is recs = 16/S (fp16-range safe); compensate by
    # scaling the ctx-rows of the projection weights by 1/16.
    wh[H:] *= np.float32(1.0 / 16.0)
    wt[H:] *= np.float32(1.0 / 16.0)
    wh16 = np.ascontiguousarray(wh, dtype=np.float16)
    wt16 = np.ascontiguousarray(wt, dtype=np.float16)
    pro = np.asarray(prototypes, dtype=np.float32)
    pos = np.asarray(mention_pos)

    in_maps = []
    for c in range(NCORES):
        b, q = divmod(c, Q)
        p_bq = pos[b, q]                       # [E, M]
        # attention gather + mention-sum: [NH, E, L] (scale dropped)
        g = att[b, q][:, p_bq, :]              # [NH, E, M, L]
        asum = g[:, :, 0, :] + g[:, :, 1, :]   # [NH, E, L]
        at = np.ascontiguousarray(
            asum.reshape(NH * E, L).T, dtype=np.float16)  # [L, NH*E]
        # normalizer S[e,f] = sum_{h,l} At[l,h,e] At[l,h,f]
        Bm = np.ascontiguousarray(
            asum.transpose(1, 0, 2).reshape(E, NH * L))
        S = Bm @ Bm.T                           # [E, E]
        recs = np.ascontiguousarray(
            (np.float32(16.0) / S).reshape(1, EF), dtype=np.float16)
        # entity means: [E, H] -> entT [H, E]
        ment = seq[b, q][p_bq]                 # [E, M, H]
        ent = (ment[:, 0, :] + ment[:, 1, :]) * np.float32(0.5)
        entT = np.ascontiguousarray(ent.T, dtype=np.float16)
        ptT = np.ascontiguousarray(
            pro[b].reshape(RP, 2 * H).T, dtype=np.float16)  # [2H, RP]
        in_maps.append({
            "at": at,
            "seq": np.ascontiguousarray(seq[b, q], dtype=np.float16),
            "entT": entT,
            "wh": wh16,
            "wt": wt16,
            "ptT": ptT,
            "recs": recs,
        })
    return in_maps


def kernel(sequence_output, attention, W_head, W_tail, prototypes,
           mention_pos):
    from concourse.bass_utils import run_bass_kernel_spmd

    if "nc" not in _CACHE:
        _CACHE["nc"] = _build_program()
    nc = _CACHE["nc"]

    in_maps = _host_prep(sequence_output, attention, W_head, W_tail,
                         prototypes, mention_pos)
    res = run_bass_kernel_spmd(nc, in_maps, core_ids=list(range(NCORES)))

    out = np.empty((B, Q, E, E, R), dtype=np.float32)
    for c in range(NCORES):
        b, q = divmod(c, Q)
        out[b, q] = res.results[c]["out"].reshape(E, E, R)
    return out
